# revision 8
# baseline (speedup 1.0000x reference)
"""CrossAttention kernel for 8 TRN2 NeuronCores (data-parallel over batch).

Reference computation (per batch element b):
  q = Wq @ x_flat                  # [512, 4096]   x_flat = x[b].reshape(512, 4096)
  kv = Wkv @ context[b].T          # [1024, 256]
  per head h (8 heads x 64 dim):
    simT_h = (k_h * SCALE).T q_h   # [256, 4096]  (j=context pos in partitions)
    E = exp(simT_h)                # softmax without max-subtract (|sim| small)
    S_h[i] = sum_j E[j, i]
    out_h = (v_h.T @ E) / S_h      # [64, 4096]
  final = Wout @ outcat + bout + x_flat

Layouts on device (per core, one batch element):
  All matmuls fp32r (1 cycle/row at N>=256).  i (pixel) chunked by 512.
"""

import numpy as np

import concourse.bass as bass
import concourse.mybir as mybir
import concourse.tile as tile
from concourse import bacc
from concourse.bass_utils import run_bass_kernel_spmd

HEADS = 8
DIM_HEAD = 64
SCALE = DIM_HEAD ** -0.5
DIM = 512          # channels of x
CTX_DIM = 768
N_CTX = 256        # context positions
HW = 4096          # 64*64 pixels
CH = 512           # i-chunk size
NCHUNK = HW // CH  # 8
B = 8              # batch == number of cores

F32 = mybir.dt.float32
F32R = mybir.dt.float32r
BF16 = mybir.dt.bfloat16


def build_bass():
    nc = bacc.Bacc(
        "TRN2",
        target_bir_lowering=False,
        debug=False,
        num_devices=B,
    )

    # DRAM parameters (per-core shard shapes)
    x_d = nc.declare_dram_parameter("x", [DIM, HW], F32R, isOutput=False)
    ctxT_d = nc.declare_dram_parameter("ctxT", [CTX_DIM, N_CTX], F32R, isOutput=False)
    wqT_d = nc.declare_dram_parameter("wqT", [DIM, DIM], F32R, isOutput=False)
    wkT_d = nc.declare_dram_parameter("wkT", [CTX_DIM, DIM], F32R, isOutput=False)
    wvT_d = nc.declare_dram_parameter("wvT", [CTX_DIM, DIM], F32R, isOutput=False)
    woutT_d = nc.declare_dram_parameter("woutT", [DIM, DIM], F32R, isOutput=False)
    bout_d = nc.declare_dram_parameter("bout2d", [128, 4], F32, isOutput=False)
    out_d = nc.declare_dram_parameter("out", [DIM, HW], F32, isOutput=True)

    # DRAM views tiled to 128 partitions
    x_t = x_d[:].rearrange("(t p) i -> p t i", p=128)        # [128, 4, 4096]
    ctxT_t = ctxT_d[:].rearrange("(t p) n -> p t n", p=128)  # [128, 6, 256]
    wqT_t = wqT_d[:].rearrange("(t p) e -> p t e", p=128)    # [128, 4, 512]
    wkT_t = wkT_d[:].rearrange("(t p) e -> p t e", p=128)    # [128, 6, 512]
    wvT_t = wvT_d[:].rearrange("(t p) e -> p t e", p=128)    # [128, 6, 512]
    woutT_t = woutT_d[:].rearrange("(t p) c -> p t c", p=128)  # [128, 4, 512]
    out_t = out_d[:].rearrange("(t p) i -> p t i", p=128)    # [128, 4, 4096]

    with tile.TileContext(nc) as tc:
        with (
            tc.tile_pool(name="wts", bufs=1) as wts,
            tc.tile_pool(name="kv", bufs=1) as kvp,
            tc.tile_pool(name="xp", bufs=3) as xp,
            tc.tile_pool(name="qp", bufs=2) as qp,
            tc.tile_pool(name="ep", bufs=3) as ep,
            tc.tile_pool(name="rp", bufs=3) as rp,
            tc.tile_pool(name="ocp", bufs=2) as ocp,
            tc.tile_pool(name="outp", bufs=2) as outp,
            tc.tile_pool(name="ps", bufs=8, space="PSUM") as ps,
        ):
            # ---- load weights / context ----
            wq_sb = wts.tile([128, 4, DIM], F32R)
            nc.gpsimd.dma_start(out=wq_sb, in_=wqT_t)
            wk_sb = wts.tile([128, 6, DIM], F32R)
            nc.gpsimd.dma_start(out=wk_sb, in_=wkT_t)
            wv_sb = wts.tile([128, 6, DIM], F32R)
            nc.gpsimd.dma_start(out=wv_sb, in_=wvT_t)
            wo_sb = wts.tile([128, 4, DIM], F32R)
            nc.gpsimd.dma_start(out=wo_sb, in_=woutT_t)
            ctx_sb = wts.tile([128, 6, N_CTX], F32R)
            nc.gpsimd.dma_start(out=ctx_sb, in_=ctxT_t)
            bout_sb = wts.tile([128, 4], F32)
            nc.gpsimd.dma_start(out=bout_sb, in_=bout_d[:])
            ones_sb = wts.tile([128, DIM_HEAD], BF16)
            nc.vector.memset(ones_sb, 1.0)

            # ---- kT = WkT.T @ ctxT : [512, 256] as [128, 4, 256] ----
            kT_sb = kvp.tile([128, 4, N_CTX], F32R)
            for m in range(4):
                pt = ps.tile([128, CH], F32, tag="ps")
                for k in range(6):
                    nc.tensor.matmul(
                        pt[:, :N_CTX],
                        (wk_sb[:, k, bass.ts(m, 128)]),
                        (ctx_sb[:, k, :]),
                        start=(k == 0),
                        stop=(k == 5),
                    )
                nc.scalar.copy(out=kT_sb[:, m, :], in_=pt[:, :N_CTX])

            # ---- v = ctxT.T @ WvT : [256, 512] as [128, 2, 512] ----
            v_sb = kvp.tile([128, 2, DIM], BF16)
            for m in range(2):
                pt = ps.tile([128, CH], F32, tag="ps")
                for k in range(6):
                    nc.tensor.matmul(
                        pt,
                        (ctx_sb[:, k, bass.ts(m, 128)]),
                        (wv_sb[:, k, :]),
                        start=(k == 0),
                        stop=(k == 5),
                    )
                nc.scalar.copy(out=v_sb[:, m, :], in_=pt)

            # ---- main loop over pixel chunks ----
            for c in range(NCHUNK):
                isl = bass.ts(c, CH)

                x_sb = xp.tile([128, 4, CH], F32R)
                nc.gpsimd.dma_start(out=x_sb, in_=x_t[:, :, isl])

                # q = WqT.T @ x  -> [128, 4, CH] (e tiles)
                q_sb = qp.tile([128, 4, CH], F32R)
                for m in range(4):
                    pt = ps.tile([128, CH], F32, tag="ps")
                    for k in range(4):
                        nc.tensor.matmul(
                            pt,
                            (wq_sb[:, k, bass.ts(m, 128)]),
                            (x_sb[:, k, :]),
                            start=(k == 0),
                            stop=(k == 3),
                        )
                    nc.scalar.copy(out=q_sb[:, m, :], in_=pt)

                # per head-pair attention
                oc_sb = ocp.tile([128, 4, CH], F32R)
                for p in range(4):  # head pair p -> heads 2p, 2p+1
                    e_tiles = []
                    for hh in range(2):  # half: head 2p+hh at partitions hh*64..
                        h0 = hh * 64
                        # simT [128(j), CH] x2 j-tiles, K=64
                        e_sb = ep.tile([128, 2, CH], BF16, tag="e")
                        for j in range(2):
                            pt = ps.tile([128, CH], F32, tag="ps")
                            nc.tensor.matmul(
                                pt,
                                (kT_sb[h0:h0 + 64, p, bass.ts(j, 128)]),
                                (q_sb[h0:h0 + 64, p, :]),
                                start=True,
                                stop=True,
                            )
                            nc.scalar.activation(
                                out=e_sb[:, j, :],
                                in_=pt,
                                func=mybir.ActivationFunctionType.Exp,
                            )
                        e_tiles.append(e_sb)

                    # attn @ v (M=64 per head, col groups 0-1 / 2-3)
                    pav = ps.tile([128, CH], F32, tag="ps")
                    pS = ps.tile([128, CH], F32, tag="ps")
                    for hh in range(2):
                        h = 2 * p + hh
                        h0 = hh * 64
                        for kj in range(2):
                            nc.tensor.matmul(
                                pav[h0:h0 + 64, :],
                                (v_sb[:, kj, bass.ds(h * 64, 64)]),
                                (e_tiles[hh][:, kj, :]),
                                start=(kj == 0),
                                stop=(kj == 1),
                            )
                    # S broadcast to 64 partitions via ones-matmul
                    for hh in range(2):
                        h0 = hh * 64
                        for kj in range(2):
                            nc.tensor.matmul(
                                pS[h0:h0 + 64, :],
                                (ones_sb),
                                (e_tiles[hh][:, kj, :]),
                                start=(kj == 0),
                                stop=(kj == 1),
                            )
                    # normalize: outcat = pav / pS
                    r_sb = rp.tile([128, CH], F32, tag="r")
                    nc.vector.reciprocal_approx_fast(out=r_sb, in_=pS)
                    nc.vector.tensor_mul(out=oc_sb[:, p, :], in0=pav, in1=r_sb)

                # out projection + bias + residual
                o_sb = outp.tile([128, 4, CH], F32)
                for m in range(4):
                    pt = ps.tile([128, CH], F32, tag="ps")
                    for k in range(4):
                        nc.tensor.matmul(
                            pt,
                            (wo_sb[:, k, bass.ts(m, 128)]),
                            (oc_sb[:, k, :]),
                            start=(k == 0),
                            stop=(k == 3),
                        )
                    nc.vector.tensor_scalar_add(
                        out=o_sb[:, m, :],
                        in0=pt,
                        scalar1=bout_sb[:, m:m + 1],
                    )
                    nc.vector.tensor_add(
                        out=o_sb[:, m, :],
                        in0=o_sb[:, m, :],
                        in1=x_sb[:, m, :].bitcast(F32),
                    )
                nc.gpsimd.dma_start(out=out_t[:, :, isl], in_=o_sb)

    nc.compile()
    return nc


_NC_CACHE = None


def _get_nc():
    global _NC_CACHE
    if _NC_CACHE is None:
        _NC_CACHE = build_bass()
    return _NC_CACHE


def make_in_maps(x, context, Wq, Wkv, Wout, bout):
    """Host-side prep: shard over batch, pre-transpose weights."""
    f = np.float32
    wqT = np.ascontiguousarray(Wq.T, dtype=f)
    wkT = np.ascontiguousarray(Wkv[:512].T * np.float32(SCALE), dtype=f)
    wvT = np.ascontiguousarray(Wkv[512:].T, dtype=f)
    woutT = np.ascontiguousarray(Wout.T, dtype=f)
    bout2d = np.ascontiguousarray(np.asarray(bout, dtype=f).reshape(4, 128).T)
    in_maps = []
    for b in range(B):
        in_maps.append({
            "x": np.ascontiguousarray(x[b].reshape(DIM, HW), dtype=f),
            "ctxT": np.ascontiguousarray(context[b].T, dtype=f),
            "wqT": wqT,
            "wkT": wkT,
            "wvT": wvT,
            "woutT": woutT,
            "bout2d": bout2d,
        })
    return in_maps


def kernel(x, context, Wq, Wkv, Wout, bout):
    x = np.asarray(x)
    context = np.asarray(context)
    nc = _get_nc()
    in_maps = make_in_maps(x, context, np.asarray(Wq), np.asarray(Wkv),
                           np.asarray(Wout), np.asarray(bout))
    res = run_bass_kernel_spmd(nc, in_maps, core_ids=list(range(B)))
    out = np.stack([res.results[b]["out"] for b in range(B)], axis=0)
    return out.reshape(B, DIM, 64, 64).astype(np.float32)


# revision 9
# speedup vs baseline: 306.9055x; 306.9055x over previous
"""CrossAttention kernel for 8 TRN2 NeuronCores (data-parallel over batch).

Reference computation (per batch element b):
  q = Wq @ x_flat                  # [512, 4096]   x_flat = x[b].reshape(512, 4096)
  kv = Wkv @ context[b].T          # [1024, 256]
  per head h (8 heads x 64 dim):
    simT_h = (k_h * SCALE).T q_h   # [256, 4096]  (j=context pos in partitions)
    E = exp(simT_h)                # softmax without max-subtract (|sim| small)
    S_h[i] = sum_j E[j, i]
    out_h = (v_h.T @ E) / S_h      # [64, 4096]
  final = Wout @ outcat + bout + x_flat

Matmuls run fp32r (1 cycle/row at N>=256); attn@v + softmax-sum run bf16
(fp32r forbids col tile_position).  i (pixel) chunked by CH=512.
"""

import numpy as np

import concourse.bass as bass
import concourse.mybir as mybir
import concourse.tile as tile
from concourse import bacc
from concourse.bass_utils import run_bass_kernel_spmd

HEADS = 8
DIM_HEAD = 64
SCALE = DIM_HEAD ** -0.5
DIM = 512          # channels of x
CTX_DIM = 768
N_CTX = 256        # context positions
HW = 4096          # 64*64 pixels
CH = 512           # i-chunk size
NCHUNK = HW // CH  # 8
B = 8              # batch == number of cores

F32 = mybir.dt.float32
F32R = mybir.dt.float32r
BF16 = mybir.dt.bfloat16


def build_bass(loop_n=1):
    nc = bacc.Bacc(
        "TRN2",
        target_bir_lowering=False,
        debug=False,
        num_devices=B,
    )

    # DRAM parameters (per-core shard shapes)
    x_d = nc.declare_dram_parameter("x", [DIM, HW], F32R, isOutput=False)
    ctxT_d = nc.declare_dram_parameter("ctxT", [CTX_DIM, N_CTX], F32R, isOutput=False)
    wqT_d = nc.declare_dram_parameter("wqT", [DIM, DIM], F32R, isOutput=False)
    wkT_d = nc.declare_dram_parameter("wkT", [CTX_DIM, DIM], F32R, isOutput=False)
    wvT_d = nc.declare_dram_parameter("wvT", [CTX_DIM, DIM], F32R, isOutput=False)
    woutT_d = nc.declare_dram_parameter("woutT", [DIM, DIM], F32R, isOutput=False)
    bout_d = nc.declare_dram_parameter("bout2d", [128, 4], F32, isOutput=False)
    out_d = nc.declare_dram_parameter("out", [DIM, HW], F32, isOutput=True)

    # DRAM views tiled to 128 partitions
    x_t = x_d[:].rearrange("(t p) i -> p t i", p=128)        # [128, 4, 4096]
    ctxT_t = ctxT_d[:].rearrange("(t p) n -> p t n", p=128)  # [128, 6, 256]
    wqT_t = wqT_d[:].rearrange("(t p) e -> p t e", p=128)    # [128, 4, 512]
    wkT_t = wkT_d[:].rearrange("(t p) e -> p t e", p=128)    # [128, 6, 512]
    wvT_t = wvT_d[:].rearrange("(t p) e -> p t e", p=128)    # [128, 6, 512]
    woutT_t = woutT_d[:].rearrange("(t p) c -> p t c", p=128)  # [128, 4, 512]
    out_t = out_d[:].rearrange("(t p) i -> p t i", p=128)    # [128, 4, 4096]

    with tile.TileContext(nc) as tc:
        with (
            tc.tile_pool(name="wts", bufs=1) as wts,
            tc.tile_pool(name="kv", bufs=1) as kvp,
            tc.tile_pool(name="xp", bufs=3) as xp,
            tc.tile_pool(name="qp", bufs=2) as qp,
            tc.tile_pool(name="ep", bufs=3) as ep,
            tc.tile_pool(name="rp", bufs=3) as rp,
            tc.tile_pool(name="ocp", bufs=2) as ocp,
            tc.tile_pool(name="outp", bufs=2) as outp,
            tc.tile_pool(name="ps", bufs=8, space="PSUM") as ps,
        ):
            # ---- load weights / context ----
            wq_sb = wts.tile([128, 4, DIM], F32R)
            nc.gpsimd.dma_start(out=wq_sb, in_=wqT_t)
            wk_sb = wts.tile([128, 6, DIM], F32R)
            nc.gpsimd.dma_start(out=wk_sb, in_=wkT_t)
            wv_sb = wts.tile([128, 6, DIM], F32R)
            nc.gpsimd.dma_start(out=wv_sb, in_=wvT_t)
            wo_sb = wts.tile([128, 4, DIM], F32R)
            nc.gpsimd.dma_start(out=wo_sb, in_=woutT_t)
            ctx_sb = wts.tile([128, 6, N_CTX], F32R)
            nc.gpsimd.dma_start(out=ctx_sb, in_=ctxT_t)
            bout_sb = wts.tile([128, 4], F32)
            nc.gpsimd.dma_start(out=bout_sb, in_=bout_d[:])
            ones_sb = wts.tile([128, DIM_HEAD], BF16)
            nc.vector.memset(ones_sb, 1.0)

            # loop_n > 1 repeats the whole compute for slope-based timing
            for _it in range(loop_n):
                # ---- kT = WkT.T @ ctxT : [512, 256] as [128, 4, 256] ----
                kT_sb = kvp.tile([128, 4, N_CTX], F32R, tag="kT")
                for m in range(4):
                    pt = ps.tile([128, CH], F32, tag="ps")
                    for k in range(6):
                        nc.tensor.matmul(
                            pt[:, :N_CTX],
                            wk_sb[:, k, bass.ts(m, 128)],
                            ctx_sb[:, k, :],
                            start=(k == 0),
                            stop=(k == 5),
                        )
                    nc.scalar.copy(out=kT_sb[:, m, :], in_=pt[:, :N_CTX])

                # ---- v = ctxT.T @ WvT : [256, 512] as [128, 2, 512] ----
                v_sb = kvp.tile([128, 2, DIM], BF16, tag="v")
                for m in range(2):
                    pt = ps.tile([128, CH], F32, tag="ps")
                    for k in range(6):
                        nc.tensor.matmul(
                            pt,
                            ctx_sb[:, k, bass.ts(m, 128)],
                            wv_sb[:, k, :],
                            start=(k == 0),
                            stop=(k == 5),
                        )
                    nc.scalar.copy(out=v_sb[:, m, :], in_=pt)

                # ---- main loop over pixel chunks ----
                for c in range(NCHUNK):
                    isl = bass.ts(c, CH)

                    x_sb = xp.tile([128, 4, CH], F32R)
                    nc.gpsimd.dma_start(out=x_sb, in_=x_t[:, :, isl])

                    # q = WqT.T @ x  -> [128, 4, CH] (e tiles)
                    q_sb = qp.tile([128, 4, CH], F32R)
                    for m in range(4):
                        pt = ps.tile([128, CH], F32, tag="ps")
                        for k in range(4):
                            nc.tensor.matmul(
                                pt,
                                wq_sb[:, k, bass.ts(m, 128)],
                                x_sb[:, k, :],
                                start=(k == 0),
                                stop=(k == 3),
                            )
                        nc.scalar.copy(out=q_sb[:, m, :], in_=pt)

                    # per head-pair attention
                    oc_sb = ocp.tile([128, 4, CH], F32R)
                    for p in range(4):  # head pair p -> heads 2p, 2p+1
                        e_tiles = []
                        for hh in range(2):  # head 2p+hh at partitions hh*64..
                            h0 = hh * 64
                            # simT [128(j), CH] x2 j-tiles, K=64
                            e_sb = ep.tile([128, 2, CH], BF16, tag="e")
                            for j in range(2):
                                pt = ps.tile([128, CH], F32, tag="ps")
                                nc.tensor.matmul(
                                    pt,
                                    kT_sb[h0:h0 + 64, p, bass.ts(j, 128)],
                                    q_sb[h0:h0 + 64, p, :],
                                    start=True,
                                    stop=True,
                                )
                                nc.scalar.activation(
                                    out=e_sb[:, j, :],
                                    in_=pt,
                                    func=mybir.ActivationFunctionType.Exp,
                                )
                            e_tiles.append(e_sb)

                        # attn @ v (M=64 per head, col groups 0-1 / 2-3)
                        pav = ps.tile([128, CH], F32, tag="ps")
                        pS = ps.tile([128, CH], F32, tag="ps")
                        for hh in range(2):
                            h = 2 * p + hh
                            h0 = hh * 64
                            for kj in range(2):
                                nc.tensor.matmul(
                                    pav[h0:h0 + 64, :],
                                    v_sb[:, kj, bass.ds(h * 64, 64)],
                                    e_tiles[hh][:, kj, :],
                                    start=(kj == 0),
                                    stop=(kj == 1),
                                )
                        # S broadcast to 64 partitions via ones-matmul
                        for hh in range(2):
                            h0 = hh * 64
                            for kj in range(2):
                                nc.tensor.matmul(
                                    pS[h0:h0 + 64, :],
                                    ones_sb,
                                    e_tiles[hh][:, kj, :],
                                    start=(kj == 0),
                                    stop=(kj == 1),
                                )
                        # normalize: outcat = pav / pS
                        r_sb = rp.tile([128, CH], F32, tag="r")
                        nc.vector.reciprocal_approx_fast(out=r_sb, in_=pS)
                        nc.vector.tensor_mul(out=oc_sb[:, p, :], in0=pav, in1=r_sb)

                    # out projection + bias + residual
                    o_sb = outp.tile([128, 4, CH], F32)
                    for m in range(4):
                        pt = ps.tile([128, CH], F32, tag="ps")
                        for k in range(4):
                            nc.tensor.matmul(
                                pt,
                                wo_sb[:, k, bass.ts(m, 128)],
                                oc_sb[:, k, :],
                                start=(k == 0),
                                stop=(k == 3),
                            )
                        nc.vector.tensor_scalar_add(
                            out=o_sb[:, m, :],
                            in0=pt,
                            scalar1=bout_sb[:, m:m + 1],
                        )
                        nc.vector.tensor_add(
                            out=o_sb[:, m, :],
                            in0=o_sb[:, m, :],
                            in1=x_sb[:, m, :].bitcast(F32),
                        )
                    nc.gpsimd.dma_start(out=out_t[:, :, isl], in_=o_sb)

    nc.compile()
    return nc


_NC_CACHE = None


def _get_nc():
    global _NC_CACHE
    if _NC_CACHE is None:
        _NC_CACHE = build_bass()
    return _NC_CACHE


def make_in_maps(x, context, Wq, Wkv, Wout, bout):
    """Host-side prep: shard over batch, pre-transpose weights."""
    f = np.float32
    wqT = np.ascontiguousarray(Wq.T, dtype=f)
    wkT = np.ascontiguousarray(Wkv[:512].T * np.float32(SCALE), dtype=f)
    wvT = np.ascontiguousarray(Wkv[512:].T, dtype=f)
    woutT = np.ascontiguousarray(Wout.T, dtype=f)
    bout2d = np.ascontiguousarray(np.asarray(bout, dtype=f).reshape(4, 128).T)
    in_maps = []
    for b in range(B):
        in_maps.append({
            "x": np.ascontiguousarray(x[b].reshape(DIM, HW), dtype=f),
            "ctxT": np.ascontiguousarray(context[b].T, dtype=f),
            "wqT": wqT,
            "wkT": wkT,
            "wvT": wvT,
            "woutT": woutT,
            "bout2d": bout2d,
        })
    return in_maps


def kernel(x, context, Wq, Wkv, Wout, bout):
    x = np.asarray(x)
    context = np.asarray(context)
    nc = _get_nc()
    in_maps = make_in_maps(x, context, np.asarray(Wq), np.asarray(Wkv),
                           np.asarray(Wout), np.asarray(bout))
    res = run_bass_kernel_spmd(nc, in_maps, core_ids=list(range(B)))
    out = np.stack([res.results[b]["out"] for b in range(B)], axis=0)
    return out.reshape(B, DIM, 64, 64).astype(np.float32)
